# revision 50
# baseline (speedup 1.0000x reference)
"""Single-head attention with QKV projections on 8 TRN2 NeuronCores.

Problem: B=4, S=2048, E=A=1024 f32.
  q = query @ Wq + bq ; k = key @ Wk + bk ; v = value @ Wv + bv
  out = softmax(q k^T / sqrt(A)) v

The v8 rewrite (projection folding, collective-free): softmax is invariant
to per-row score offsets, so with W_qk := Wq @ Wk^T (host f32 GEMM, one
bf16 rounding instead of two) and bqk := bq @ Wk^T,
  scores = (query W_qk + bqk) key^T   [+ per-row terms that cancel]
EXACTLY reproduces softmax((query Wq + bq)(key Wk + bk)^T): the dropped
terms (query Wq bk^T and bq bk^T) are constant along each key row. The
V projection re-associates: out = (probs @ value) @ Wv + bv (sum probs = 1
makes bv additive after the @Wv). So the per-core work is
  QP: qT = (query W_qk + bqk)^T            128 matmuls
  S : exp(qT^T key^T / 32) -> E_t          256 matmuls
  AV1: yT = value^T @ E_t                  256 matmuls
  AV2: out = (yT^T @ Wv) * recip + bv      128 matmuls
768 128x128x512-bf16 matmuls/core (vs 896 with on-device K/V projection)
plus 16 tiny denominator matmuls -- and NO collectives: sharding is purely
data-parallel over (batch, query-half); every core receives its batch's
full keyT/value images from the host, so the K/V-projection dedup
AllGathers (and their CC rendezvous + mesh HBM traffic) disappear.

DMA discipline (the v5 lesson): HWDGE descriptor generation costs ~5.4ns
per contiguous segment, so all inputs are host-supplied in the exact SBUF
image layout (contiguous per-partition rows), pre-chunked so every load's
destination is contiguous.

Input streaming (the v7 lesson): the ~14MB/core input prefetch is
HBM-bandwidth-bound (~0.17-0.36 MB/us/core with 8 cores pulling), so the
whole stream rides ONE queue (Scalar) in exact phase-consumption order
(wqk at-major 256KB blocks + xq halves, then keyT in 512-key chunks,
value in 4-kt chunks, wv halves): each phase's first matmul waits only
for its first few hundred KB, and the last-arriving bytes are also the
last-needed. Sync carries only the output stores; gpsimd only the tiny
bias loads. PE warm-up matmuls cover the ~8us DMA-ring boot + first-input
window so the HAM clock gate is at 2.4GHz when QP starts.

All matmul operands bf16 (PSUM f32). No row-max subtraction before exp:
scores ~ N(0,1), |scores| <= ~6. Measured rel_l2 ~5e-3 (tolerance 2e-2).
"""
import sys

sys.path.insert(0, "/opt/trn_rl_repo")

import ml_dtypes
import numpy as np

BF16 = ml_dtypes.bfloat16

import concourse.bass as bass
import concourse.tile as tile
from concourse import bacc, bass_utils, mybir

B, S, E, A = 4, 2048, 1024, 1024
SQ = 1024          # queries per core
ET, AT = 8, 8      # 128-tiles of E and A
KT = 16            # 128-tiles of the 2048 keys
KC = 4             # 512-key chunks
QC, QS, AC = 2, 8, 2    # q 512-chunks, q 128-subtiles, a 512-chunks
SCALE = 1.0 / 32.0      # 1/sqrt(A)
NWARM = 20              # PE warm-up matmuls during the initial DMA window

f32 = mybir.dt.float32
bf16 = mybir.dt.bfloat16
ts = bass.ts


def build():
    nc = bacc.Bacc("TRN2", target_bir_lowering=False, debug=False,
                   dynamic_dma_scratch_size=8192)
    Act = mybir.ActivationFunctionType
    Alu = mybir.AluOpType

    # Host-supplied SBUF images (contiguous per-partition rows):
    #   xq_d  [128, et*1024]: xq[p, et*SQ+j]   = query_half^T[et*128+p, j]
    #   wqk_d [128, at*1024]: at-major blocks, wqk[p, at*1024 + et*128 + j]
    #                         = W_qk[et*128+p, at*128+j]
    #   kt_d  [128, kc*4096]: key-chunk-major, kt_d[p, kc*4096 + et*512 + j]
    #                         = key_b^T[et*128+p, kc*512+j]
    #   xv_d  [128, kt*1024]: value rows tiled by key, xv_d[p, kt*1024+e]
    #                         = value_b[kt*128+p, e]
    #   wv_d  [128, et*1024]: wv_d[p, et*1024+j] = Wv[et*128+p, j]
    xq_d = nc.dram_tensor("xq", [128, ET * SQ], bf16, kind="ExternalInput")
    wqk_d = nc.dram_tensor("wqk", [128, AT * SQ], bf16, kind="ExternalInput")
    kt_d = nc.dram_tensor("ktc", [128, KC * ET * 512], bf16,
                          kind="ExternalInput")
    xv_d = nc.dram_tensor("xv", [128, KT * A], bf16, kind="ExternalInput")
    wv_d = nc.dram_tensor("wv", [128, ET * A], bf16, kind="ExternalInput")
    bqkt_d = nc.dram_tensor("bqkt", [128, AT], f32, kind="ExternalInput")
    bvb_d = nc.dram_tensor("bvb", [128, A], f32, kind="ExternalInput")
    ones_d = nc.dram_tensor("ones", [128, 2], f32, kind="ExternalInput")
    out_d = nc.dram_tensor("out", [SQ, A], f32, kind="ExternalOutput")

    # Long-lived activations as raw (non-pool) SBUF tensors.
    qT = nc.alloc_sbuf_tensor("qT_sb", [128, ET, SQ], bf16).ap()
    kT = nc.alloc_sbuf_tensor("kT_sb", [128, KC, ET, 512], bf16).ap()
    v_sb = nc.alloc_sbuf_tensor("v_sb", [128, KT, A], bf16).ap()
    yT = nc.alloc_sbuf_tensor("yT_sb", [128, ET, SQ], bf16).ap()
    acc = nc.alloc_sbuf_tensor("acc_sb", [128, SQ], f32).ap()
    recip = nc.alloc_sbuf_tensor("recip_sb", [128, QS], f32).ap()
    ones_t = nc.alloc_sbuf_tensor("ones_sb", [128, 2], f32).ap()

    with tile.TileContext(nc) as tc:
        with (
            tc.tile_pool(name="pp512", bufs=2, space="PSUM") as pp512,
            tc.tile_pool(name="pps", bufs=3, space="PSUM") as pps,
        ):
            pe = tc.alloc_tile_pool(name="pe", bufs=1)
            E_t = pe.tile([128, KT, SQ], bf16)  # exp(scores^T) [k, kt, q]
            pwq = tc.alloc_tile_pool(name="pwq", bufs=1)
            pwv = tc.alloc_tile_pool(name="pwv", bufs=1)
            pxq = tc.alloc_tile_pool(name="pxq", bufs=1)
            pcs = tc.alloc_tile_pool(name="pcs", bufs=1)
            pot = tc.alloc_tile_pool(name="pot", bufs=3)

            # ---- PE warm-up: dummy matmuls on a zeroed SBUF scrap while
            # the DMA rings boot (~8us) and the first inputs land, so the
            # HAM clock ramp overlaps the dead startup window.
            nc.vector.memset(qT[:, 0, 0:512], 0.0)
            for i in range(NWARM):
                wps = pp512.tile([128, 512], f32, tag="ps", name="wps")
                nc.tensor.matmul(wps[:], qT[:, 0, 0:128], qT[:, 0, 0:512],
                                 start=True, stop=True)

            # ---- Input stream, ONE queue (Scalar), consumption order ----
            wqk = pwq.tile([128, AT, ET * 128], bf16)
            xq_t = pxq.tile([128, QC, ET, 512], bf16)
            wqkd = wqk_d.ap()
            nc.scalar.dma_start(wqk[:, 0, :], wqkd[:, 0:1024])
            # xq image is qc-major and qc0 arrives in two et-chunks:
            # QP's first chain starts after ~768KB of QP input
            for h in range(2):
                nc.scalar.dma_start(
                    xq_t[:, 0, ts(h, 4), :],
                    xq_d.ap()[:, h * 2048:(h + 1) * 2048].rearrange(
                        "p (et j) -> p et j", j=512))
            nc.scalar.dma_start(
                xq_t[:, 1, :, :],
                xq_d.ap()[:, 4096:8192].rearrange(
                    "p (et j) -> p et j", j=512))
            for at in range(1, AT):
                nc.scalar.dma_start(
                    wqk[:, at, :], wqkd[:, at * 1024:(at + 1) * 1024])
            for kc in range(KC):     # keyT in S-phase chunk order
                nc.scalar.dma_start(
                    kT[:, kc, :, :],
                    kt_d.ap()[:, kc * 4096:(kc + 1) * 4096].rearrange(
                        "p (et j) -> p et j", j=512))
            for h in range(4):       # value in 4-kt chunks (AV1 order)
                nc.scalar.dma_start(
                    v_sb[:, ts(h, 4), :],
                    xv_d.ap()[:, h * 4096:(h + 1) * 4096].rearrange(
                        "p (kt j) -> p kt j", j=A))
            wv = pwv.tile([128, ET, A], bf16)
            for h in range(2):
                nc.scalar.dma_start(
                    wv[:, ts(h, 4), :],
                    wv_d.ap()[:, h * 4096:(h + 1) * 4096].rearrange(
                        "p (b j) -> p b j", j=A))

            # tiny bias constants ride gpsimd (SWDGE), off both big queues
            bqkt = pcs.tile([128, AT], f32, tag="bqkt")
            nc.gpsimd.dma_start(bqkt[:], bqkt_d.ap()[:, :])
            nc.gpsimd.dma_start(ones_t[:], ones_d.ap()[:, :])
            bvb = pcs.tile([128, A], f32, tag="bvb")
            nc.gpsimd.dma_start(bvb[:], bvb_d.ap()[:, :])

            # ---- Phase QP: qT[e', q] = (query @ W_qk + bqk)^T ----
            for at in range(AT):
                ps = pps.tile([128, SQ], f32, tag="psc", name="ps_a")
                for qc in range(QC):   # qc-outer: the first 8-matmul pass
                    for et in range(ET):   # only needs xq's qc=0 half
                        nc.tensor.matmul(
                            ps[:, ts(qc, 512)], wqk[:, at, ts(et, 128)],
                            xq_t[:, qc, et, :],
                            start=(et == 0), stop=(et == ET - 1),
                        )
                nc.vector.tensor_scalar(
                    qT[:, at, :], ps[:], bqkt[:, at:at + 1], None, Alu.add)

            # ---- Phase S: scores^T = kT-tile @ qT -> exp -> E_t; Vector
            #      accumulates softmax denominators in production order ----
            nprod = 0
            for kc in range(KC):
                for ki in range(4):
                    kt = kc * 4 + ki
                    psc = pps.tile([128, SQ], f32, tag="psc", name="psc")
                    for et in range(ET):
                        for qc in range(QC):
                            nc.tensor.matmul(
                                psc[:, ts(qc, 512)],
                                kT[:, kc, et, ts(ki, 128)],
                                qT[:, et, ts(qc, 512)],
                                start=(et == 0), stop=(et == ET - 1),
                            )
                    nc.scalar.activation(
                        E_t[:, kt, :], psc[:], Act.Exp, bias=0.0,
                        scale=SCALE)
                    nprod += 1
                    if nprod == 2:
                        nc.vector.tensor_tensor(
                            acc[:], E_t[:, 0, :], E_t[:, 1, :], Alu.add)
                    elif nprod > 2:
                        nc.vector.tensor_tensor(
                            acc[:], acc[:], E_t[:, kt, :], Alu.add)

            # ---- Phase AV1: yT[e, q] = value^T @ E_t (unnormalized) ----
            for es in range(ET):
                ps = pps.tile([128, SQ], f32, tag="psc", name="ps_y")
                for kt in range(KT):
                    for qc in range(QC):
                        nc.tensor.matmul(
                            ps[:, ts(qc, 512)], v_sb[:, kt, ts(es, 128)],
                            E_t[:, kt, ts(qc, 512)],
                            start=(kt == 0), stop=(kt == KT - 1),
                        )
                nc.vector.tensor_copy(yT[:, es, :], ps[:])

            # denominators: 128-way partition reduction of acc via tiny
            # matmuls with a ones column; tucked between AV1 and AV2 so
            # the PE covers the last acc adds / yT drain
            for dq in range(QS):
                psd = pp512.tile([128, 2], f32, tag="ps", name="psd")
                nc.tensor.matmul(
                    psd[:], acc[:, ts(dq, 128)], ones_t[:],
                    start=True, stop=True)
                nc.vector.reciprocal(recip[:, dq:dq + 1], psd[:, 0:1])

            # ---- Phase AV2: out = (yT^T @ Wv) * recip + bv ----
            for qs in range(QS):
                ps = pps.tile([128, SQ], f32, tag="psc", name="ps_av")
                for et in range(ET):
                    for ac in range(AC):
                        nc.tensor.matmul(
                            ps[:, ts(ac, 512)], yT[:, et, ts(qs, 128)],
                            wv[:, et, ts(ac, 512)],
                            start=(et == 0), stop=(et == ET - 1),
                        )
                ot = pot.tile([128, SQ], f32, tag="ot", name="ot")
                nhalf = 2 if qs >= QS - 2 else 1
                step = SQ // nhalf
                for h in range(nhalf):
                    sl = slice(h * step, (h + 1) * step)
                    nc.vector.tensor_scalar(
                        ot[:, sl], ps[:, sl], recip[:, qs:qs + 1],
                        None, Alu.mult)
                    nc.vector.tensor_tensor(
                        ot[:, sl], ot[:, sl], bvb[:, sl], Alu.add)
                    nc.sync.dma_start(
                        out_d.ap()[ts(qs, 128), sl], ot[:, sl])

            for p in (pot, pcs, pxq, pwv, pwq, pe):
                p.release()

    nc.compile()
    return nc


_nc_cache = None


def _get_nc():
    global _nc_cache
    if _nc_cache is None:
        _nc_cache = build()
    return _nc_cache


def _img(xT, c0=None, c1=None):
    """[E, n] -> SBUF image [128, 8*n'] (p-major), optionally col-sliced."""
    t = xT.reshape(ET, 128, xT.shape[1]).transpose(1, 0, 2)
    if c0 is None:
        return np.ascontiguousarray(t.reshape(128, -1))
    return np.ascontiguousarray(t[:, :, c0:c1].reshape(128, -1))


def kernel(query, key, value, Wq, bq, Wk, bk, Wv, bv):
    query = np.asarray(query, dtype=np.float32)
    key = np.asarray(key, dtype=np.float32)
    value = np.asarray(value, dtype=np.float32)
    Wq = np.ascontiguousarray(np.asarray(Wq, dtype=np.float32))
    Wk = np.ascontiguousarray(np.asarray(Wk, dtype=np.float32))
    Wv = np.ascontiguousarray(np.asarray(Wv, dtype=np.float32))
    bq = np.asarray(bq, dtype=np.float32)
    bk = np.asarray(bk, dtype=np.float32)
    bv = np.asarray(bv, dtype=np.float32)

    nc = _get_nc()

    # Projection folding (see module docstring): scores row-offsets from
    # bk cancel in softmax, so only W_qk and bqk are needed.
    Wqk16 = (Wq @ Wk.T).astype(BF16)
    bqk = bq @ Wk.T                       # [E]
    Wv16 = Wv.astype(BF16)

    wqk_i = np.concatenate(
        [_img(Wqk16, at * 128, (at + 1) * 128) for at in range(AT)], axis=1)
    wv_i = _img(Wv16)
    bqkt = np.ascontiguousarray(bqk.reshape(AT, 128).T)
    bvb = np.ascontiguousarray(np.broadcast_to(bv, (128, A)))
    ones = np.ones((128, 2), np.float32)

    in_maps = []
    for c in range(8):
        b, h = c // 2, c % 2
        xqT = query[b, h * SQ:(h + 1) * SQ, :].T.astype(BF16)
        keyT = key[b].T.astype(BF16)              # [E, 2048]
        val16 = value[b].astype(BF16)             # [2048, E]
        xq_img = np.concatenate(
            [_img(xqT, qc * 512, (qc + 1) * 512) for qc in range(QC)],
            axis=1)
        kt_img = np.concatenate(
            [_img(keyT, kc * 512, (kc + 1) * 512) for kc in range(KC)],
            axis=1)
        xv_img = np.ascontiguousarray(
            val16.reshape(KT, 128, A).transpose(1, 0, 2).reshape(128, -1))
        in_maps.append({
            "xq": xq_img,
            "wqk": wqk_i,
            "ktc": kt_img,
            "xv": xv_img,
            "wv": wv_i,
            "bqkt": bqkt,
            "bvb": bvb,
            "ones": ones,
        })

    global _last_in_maps
    _last_in_maps = in_maps
    res = bass_utils.run_bass_kernel_spmd(nc, in_maps, core_ids=list(range(8)))

    out = np.empty((B, S, A), np.float32)
    for c in range(8):
        b, h = c // 2, c % 2
        out[b, h * SQ:(h + 1) * SQ, :] = res.results[c]["out"]
    return out


# revision 52
# speedup vs baseline: 1.0176x; 1.0176x over previous
"""Single-head attention with QKV projections on 8 TRN2 NeuronCores.

Problem: B=4, S=2048, E=A=1024 f32.
  q = query @ Wq + bq ; k = key @ Wk + bk ; v = value @ Wv + bv
  out = softmax(q k^T / sqrt(A)) v

The v8 rewrite (projection folding, collective-free): softmax is invariant
to per-row score offsets, so with W_qk := Wq @ Wk^T (host f32 GEMM, one
bf16 rounding instead of two) and bqk := bq @ Wk^T,
  scores = (query W_qk + bqk) key^T   [+ per-row terms that cancel]
EXACTLY reproduces softmax((query Wq + bq)(key Wk + bk)^T): the dropped
terms (query Wq bk^T and bq bk^T) are constant along each key row. The
V projection re-associates: out = (probs @ value) @ Wv + bv (sum probs = 1
makes bv additive after the @Wv). So the per-core work is
  QP: qT = (query W_qk + bqk)^T            128 matmuls
  S : exp(qT^T key^T / 32) -> E_t          256 matmuls
  AV1: yT = value^T @ E_t                  256 matmuls
  AV2: out = (yT^T @ Wv) * recip + bv      128 matmuls
768 128x128x512-bf16 matmuls/core (vs 896 with on-device K/V projection)
plus 16 tiny denominator matmuls -- and NO collectives: sharding is purely
data-parallel over (batch, query-half); every core receives its batch's
full keyT/value images from the host, so the K/V-projection dedup
AllGathers (and their CC rendezvous + mesh HBM traffic) disappear.

DMA discipline (the v5 lesson): HWDGE descriptor generation costs ~5.4ns
per contiguous segment, so all inputs are host-supplied in the exact SBUF
image layout (contiguous per-partition rows), pre-chunked so every load's
destination is contiguous.

Input streaming (the v7 lesson): the ~14MB/core input prefetch is
HBM-bandwidth-bound (~0.17-0.36 MB/us/core with 8 cores pulling), so the
whole stream rides ONE queue (Scalar) in exact phase-consumption order
(wqk at-major 256KB blocks + xq halves, then keyT in 512-key chunks,
value in 4-kt chunks, wv halves): each phase's first matmul waits only
for its first few hundred KB, and the last-arriving bytes are also the
last-needed. Sync carries only the output stores; gpsimd only the tiny
bias loads. PE warm-up matmuls cover the ~8us DMA-ring boot + first-input
window so the HAM clock gate is at 2.4GHz when QP starts.

All matmul operands bf16 (PSUM f32). No row-max subtraction before exp:
scores ~ N(0,1), |scores| <= ~6. Measured rel_l2 ~5e-3 (tolerance 2e-2).
"""
import sys

sys.path.insert(0, "/opt/trn_rl_repo")

import ml_dtypes
import numpy as np

BF16 = ml_dtypes.bfloat16

import concourse.bass as bass
import concourse.tile as tile
from concourse import bacc, bass_utils, mybir

B, S, E, A = 4, 2048, 1024, 1024
SQ = 1024          # queries per core
ET, AT = 8, 8      # 128-tiles of E and A
KT = 16            # 128-tiles of the 2048 keys
KC = 4             # 512-key chunks
QC, QS, AC = 2, 8, 2    # q 512-chunks, q 128-subtiles, a 512-chunks
SCALE = 1.0 / 32.0      # 1/sqrt(A)
NWARM = 17              # PE warm-up matmuls during the initial DMA window

f32 = mybir.dt.float32
bf16 = mybir.dt.bfloat16
ts = bass.ts


def build():
    nc = bacc.Bacc("TRN2", target_bir_lowering=False, debug=False,
                   dynamic_dma_scratch_size=8192)
    Act = mybir.ActivationFunctionType
    Alu = mybir.AluOpType

    # Host-supplied SBUF images (contiguous per-partition rows):
    #   xq_d  [128, et*1024]: xq[p, et*SQ+j]   = query_half^T[et*128+p, j]
    #   wqk_d [128, at*1024]: at-major blocks, wqk[p, at*1024 + et*128 + j]
    #                         = W_qk[et*128+p, at*128+j]
    #   kt_d  [128, kc*4096]: key-chunk-major, kt_d[p, kc*4096 + et*512 + j]
    #                         = key_b^T[et*128+p, kc*512+j]
    #   xv_d  [128, kt*1024]: value rows tiled by key, xv_d[p, kt*1024+e]
    #                         = value_b[kt*128+p, e]
    #   wv_d  [128, et*1024]: wv_d[p, et*1024+j] = Wv[et*128+p, j]
    xq_d = nc.dram_tensor("xq", [128, ET * SQ], bf16, kind="ExternalInput")
    wqk_d = nc.dram_tensor("wqk", [128, AT * SQ], bf16, kind="ExternalInput")
    kt_d = nc.dram_tensor("ktc", [128, KC * ET * 512], bf16,
                          kind="ExternalInput")
    xv_d = nc.dram_tensor("xv", [128, KT * A], bf16, kind="ExternalInput")
    wv_d = nc.dram_tensor("wv", [128, ET * A], bf16, kind="ExternalInput")
    bqkt_d = nc.dram_tensor("bqkt", [128, AT], f32, kind="ExternalInput")
    bvb_d = nc.dram_tensor("bvb", [128, A], f32, kind="ExternalInput")
    ones_d = nc.dram_tensor("ones", [128, 2], f32, kind="ExternalInput")
    out_d = nc.dram_tensor("out", [SQ, A], bf16, kind="ExternalOutput")

    # Long-lived activations as raw (non-pool) SBUF tensors.
    qT = nc.alloc_sbuf_tensor("qT_sb", [128, ET, SQ], bf16).ap()
    kT = nc.alloc_sbuf_tensor("kT_sb", [128, KC, ET, 512], bf16).ap()
    v_sb = nc.alloc_sbuf_tensor("v_sb", [128, KT, A], bf16).ap()
    yT = nc.alloc_sbuf_tensor("yT_sb", [128, ET, SQ], bf16).ap()
    acc = nc.alloc_sbuf_tensor("acc_sb", [128, SQ], f32).ap()
    recip = nc.alloc_sbuf_tensor("recip_sb", [128, QS], f32).ap()
    ones_t = nc.alloc_sbuf_tensor("ones_sb", [128, 2], f32).ap()

    with tile.TileContext(nc) as tc:
        with (
            tc.tile_pool(name="pp512", bufs=2, space="PSUM") as pp512,
            tc.tile_pool(name="pps", bufs=3, space="PSUM") as pps,
        ):
            pe = tc.alloc_tile_pool(name="pe", bufs=1)
            E_t = pe.tile([128, KT, SQ], bf16)  # exp(scores^T) [k, kt, q]
            pwq = tc.alloc_tile_pool(name="pwq", bufs=1)
            pwv = tc.alloc_tile_pool(name="pwv", bufs=1)
            pxq = tc.alloc_tile_pool(name="pxq", bufs=1)
            pcs = tc.alloc_tile_pool(name="pcs", bufs=1)
            pot = tc.alloc_tile_pool(name="pot", bufs=3)

            # ---- PE warm-up: dummy matmuls on a zeroed SBUF scrap while
            # the DMA rings boot (~8us) and the first inputs land, so the
            # HAM clock ramp overlaps the dead startup window.
            nc.vector.memset(qT[:, 0, 0:512], 0.0)
            for i in range(NWARM):
                wps = pp512.tile([128, 512], f32, tag="ps", name="wps")
                nc.tensor.matmul(wps[:], qT[:, 0, 0:128], qT[:, 0, 0:512],
                                 start=True, stop=True)

            # ---- Input stream, ONE queue (Scalar), consumption order ----
            wqk = pwq.tile([128, AT, ET * 128], bf16)
            xq_t = pxq.tile([128, QC, ET, 512], bf16)
            wqkd = wqk_d.ap()
            nc.scalar.dma_start(wqk[:, 0, :], wqkd[:, 0:1024])
            # xq image is qc-major and qc0 arrives in four 256KB
            # et-pair chunks: QP's first matmul waits for ~512KB
            for h in range(4):
                nc.scalar.dma_start(
                    xq_t[:, 0, ts(h, 2), :],
                    xq_d.ap()[:, h * 1024:(h + 1) * 1024].rearrange(
                        "p (et j) -> p et j", j=512))
            nc.scalar.dma_start(
                xq_t[:, 1, :, :],
                xq_d.ap()[:, 4096:8192].rearrange(
                    "p (et j) -> p et j", j=512))
            for at in range(1, AT):
                nc.scalar.dma_start(
                    wqk[:, at, :], wqkd[:, at * 1024:(at + 1) * 1024])
            for kc in range(KC):     # keyT in S-phase chunk order
                nc.scalar.dma_start(
                    kT[:, kc, :, :],
                    kt_d.ap()[:, kc * 4096:(kc + 1) * 4096].rearrange(
                        "p (et j) -> p et j", j=512))
            for h in range(4):       # value in 4-kt chunks (AV1 order)
                nc.scalar.dma_start(
                    v_sb[:, ts(h, 4), :],
                    xv_d.ap()[:, h * 4096:(h + 1) * 4096].rearrange(
                        "p (kt j) -> p kt j", j=A))
            wv = pwv.tile([128, ET, A], bf16)
            for h in range(2):
                nc.scalar.dma_start(
                    wv[:, ts(h, 4), :],
                    wv_d.ap()[:, h * 4096:(h + 1) * 4096].rearrange(
                        "p (b j) -> p b j", j=A))

            # tiny bias constants ride gpsimd (SWDGE), off both big queues
            bqkt = pcs.tile([128, AT], f32, tag="bqkt")
            nc.gpsimd.dma_start(bqkt[:], bqkt_d.ap()[:, :])
            nc.gpsimd.dma_start(ones_t[:], ones_d.ap()[:, :])
            bvb = pcs.tile([128, A], f32, tag="bvb")
            nc.gpsimd.dma_start(bvb[:], bvb_d.ap()[:, :])

            # ---- Phase QP: qT[e', q] = (query @ W_qk + bqk)^T ----
            for at in range(AT):
                ps = pps.tile([128, SQ], f32, tag="psc", name="ps_a")
                for qc in range(QC):   # qc-outer: the first 8-matmul pass
                    for et in range(ET):   # only needs xq's qc=0 half
                        nc.tensor.matmul(
                            ps[:, ts(qc, 512)], wqk[:, at, ts(et, 128)],
                            xq_t[:, qc, et, :],
                            start=(et == 0), stop=(et == ET - 1),
                        )
                nc.vector.tensor_scalar(
                    qT[:, at, :], ps[:], bqkt[:, at:at + 1], None, Alu.add)

            # ---- Phase S: scores^T = kT-tile @ qT -> exp -> E_t; Vector
            #      accumulates softmax denominators in production order ----
            nprod = 0
            for kc in range(KC):
                for ki in range(4):
                    kt = kc * 4 + ki
                    psc = pps.tile([128, SQ], f32, tag="psc", name="psc")
                    for et in range(ET):
                        for qc in range(QC):
                            nc.tensor.matmul(
                                psc[:, ts(qc, 512)],
                                kT[:, kc, et, ts(ki, 128)],
                                qT[:, et, ts(qc, 512)],
                                start=(et == 0), stop=(et == ET - 1),
                            )
                    nc.scalar.activation(
                        E_t[:, kt, :], psc[:], Act.Exp, bias=0.0,
                        scale=SCALE)
                    nprod += 1
                    if nprod == 2:
                        nc.vector.tensor_tensor(
                            acc[:], E_t[:, 0, :], E_t[:, 1, :], Alu.add)
                    elif nprod > 2:
                        nc.vector.tensor_tensor(
                            acc[:], acc[:], E_t[:, kt, :], Alu.add)

            # ---- Phase AV1: yT[e, q] = value^T @ E_t (unnormalized) ----
            for es in range(ET):
                ps = pps.tile([128, SQ], f32, tag="psc", name="ps_y")
                for kt in range(KT):
                    for qc in range(QC):
                        nc.tensor.matmul(
                            ps[:, ts(qc, 512)], v_sb[:, kt, ts(es, 128)],
                            E_t[:, kt, ts(qc, 512)],
                            start=(kt == 0), stop=(kt == KT - 1),
                        )
                nc.vector.tensor_copy(yT[:, es, :], ps[:])

            # denominators: 128-way partition reduction of acc via tiny
            # matmuls with a ones column; tucked between AV1 and AV2 so
            # the PE covers the last acc adds / yT drain
            for dq in range(QS):
                psd = pp512.tile([128, 2], f32, tag="ps", name="psd")
                nc.tensor.matmul(
                    psd[:], acc[:, ts(dq, 128)], ones_t[:],
                    start=True, stop=True)
                nc.vector.reciprocal(recip[:, dq:dq + 1], psd[:, 0:1])

            # ---- Phase AV2: out = (yT^T @ Wv) * recip + bv ----
            for qs in range(QS):
                ps = pps.tile([128, SQ], f32, tag="psc", name="ps_av")
                for et in range(ET):
                    for ac in range(AC):
                        nc.tensor.matmul(
                            ps[:, ts(ac, 512)], yT[:, et, ts(qs, 128)],
                            wv[:, et, ts(ac, 512)],
                            start=(et == 0), stop=(et == ET - 1),
                        )
                ot = pot.tile([128, SQ], bf16, tag="ot", name="ot")
                nhalf = 2 if qs >= QS - 2 else 1
                step = SQ // nhalf
                for h in range(nhalf):
                    sl = slice(h * step, (h + 1) * step)
                    nc.vector.tensor_scalar(
                        ot[:, sl], ps[:, sl], recip[:, qs:qs + 1],
                        None, Alu.mult)
                    nc.vector.tensor_tensor(
                        ot[:, sl], ot[:, sl], bvb[:, sl], Alu.add)
                    nc.sync.dma_start(
                        out_d.ap()[ts(qs, 128), sl], ot[:, sl])

            for p in (pot, pcs, pxq, pwv, pwq, pe):
                p.release()

    nc.compile()
    return nc


_nc_cache = None


def _get_nc():
    global _nc_cache
    if _nc_cache is None:
        _nc_cache = build()
    return _nc_cache


def _img(xT, c0=None, c1=None):
    """[E, n] -> SBUF image [128, 8*n'] (p-major), optionally col-sliced."""
    t = xT.reshape(ET, 128, xT.shape[1]).transpose(1, 0, 2)
    if c0 is None:
        return np.ascontiguousarray(t.reshape(128, -1))
    return np.ascontiguousarray(t[:, :, c0:c1].reshape(128, -1))


def kernel(query, key, value, Wq, bq, Wk, bk, Wv, bv):
    query = np.asarray(query, dtype=np.float32)
    key = np.asarray(key, dtype=np.float32)
    value = np.asarray(value, dtype=np.float32)
    Wq = np.ascontiguousarray(np.asarray(Wq, dtype=np.float32))
    Wk = np.ascontiguousarray(np.asarray(Wk, dtype=np.float32))
    Wv = np.ascontiguousarray(np.asarray(Wv, dtype=np.float32))
    bq = np.asarray(bq, dtype=np.float32)
    bk = np.asarray(bk, dtype=np.float32)
    bv = np.asarray(bv, dtype=np.float32)

    nc = _get_nc()

    # Projection folding (see module docstring): scores row-offsets from
    # bk cancel in softmax, so only W_qk and bqk are needed.
    Wqk16 = (Wq @ Wk.T).astype(BF16)
    bqk = bq @ Wk.T                       # [E]
    Wv16 = Wv.astype(BF16)

    wqk_i = np.concatenate(
        [_img(Wqk16, at * 128, (at + 1) * 128) for at in range(AT)], axis=1)
    wv_i = _img(Wv16)
    bqkt = np.ascontiguousarray(bqk.reshape(AT, 128).T)
    bvb = np.ascontiguousarray(np.broadcast_to(bv, (128, A)))
    ones = np.ones((128, 2), np.float32)

    in_maps = []
    for c in range(8):
        b, h = c // 2, c % 2
        xqT = query[b, h * SQ:(h + 1) * SQ, :].T.astype(BF16)
        keyT = key[b].T.astype(BF16)              # [E, 2048]
        val16 = value[b].astype(BF16)             # [2048, E]
        xq_img = np.concatenate(
            [_img(xqT, qc * 512, (qc + 1) * 512) for qc in range(QC)],
            axis=1)
        kt_img = np.concatenate(
            [_img(keyT, kc * 512, (kc + 1) * 512) for kc in range(KC)],
            axis=1)
        xv_img = np.ascontiguousarray(
            val16.reshape(KT, 128, A).transpose(1, 0, 2).reshape(128, -1))
        in_maps.append({
            "xq": xq_img,
            "wqk": wqk_i,
            "ktc": kt_img,
            "xv": xv_img,
            "wv": wv_i,
            "bqkt": bqkt,
            "bvb": bvb,
            "ones": ones,
        })

    global _last_in_maps
    _last_in_maps = in_maps
    res = bass_utils.run_bass_kernel_spmd(nc, in_maps, core_ids=list(range(8)))

    out = np.empty((B, S, A), np.float32)
    for c in range(8):
        b, h = c // 2, c % 2
        out[b, h * SQ:(h + 1) * SQ, :] = np.asarray(
            res.results[c]["out"], dtype=np.float32)
    return out


# revision 53
# speedup vs baseline: 1.0227x; 1.0050x over previous
"""Single-head attention with QKV projections on 8 TRN2 NeuronCores.

Problem: B=4, S=2048, E=A=1024 f32.
  q = query @ Wq + bq ; k = key @ Wk + bk ; v = value @ Wv + bv
  out = softmax(q k^T / sqrt(A)) v

The v8 rewrite (projection folding, collective-free): softmax is invariant
to per-row score offsets, so with W_qk := Wq @ Wk^T (host f32 GEMM, one
bf16 rounding instead of two) and bqk := bq @ Wk^T,
  scores = (query W_qk + bqk) key^T   [+ per-row terms that cancel]
EXACTLY reproduces softmax((query Wq + bq)(key Wk + bk)^T): the dropped
terms (query Wq bk^T and bq bk^T) are constant along each key row. The
V projection re-associates: out = (probs @ value) @ Wv + bv (sum probs = 1
makes bv additive after the @Wv). So the per-core work is
  QP: qT = (query W_qk + bqk)^T            128 matmuls
  S : exp(qT^T key^T / 32) -> E_t          256 matmuls
  AV1: yT = value^T @ E_t                  256 matmuls
  AV2: out = (yT^T @ Wv) * recip + bv      128 matmuls
768 128x128x512-bf16 matmuls/core (vs 896 with on-device K/V projection)
plus 16 tiny denominator matmuls -- and NO collectives: sharding is purely
data-parallel over (batch, query-half); every core receives its batch's
full keyT/value images from the host, so the K/V-projection dedup
AllGathers (and their CC rendezvous + mesh HBM traffic) disappear.

DMA discipline (the v5 lesson): HWDGE descriptor generation costs ~5.4ns
per contiguous segment, so all inputs are host-supplied in the exact SBUF
image layout (contiguous per-partition rows), pre-chunked so every load's
destination is contiguous.

Input streaming (the v7 lesson): the ~14MB/core input prefetch is
HBM-bandwidth-bound (~0.17-0.36 MB/us/core with 8 cores pulling), so the
whole stream rides ONE queue (Scalar) in exact phase-consumption order
(wqk at-major 256KB blocks + xq halves, then keyT in 512-key chunks,
value in 4-kt chunks, wv halves): each phase's first matmul waits only
for its first few hundred KB, and the last-arriving bytes are also the
last-needed. Sync carries only the output stores; gpsimd only the tiny
bias loads. PE warm-up matmuls cover the ~8us DMA-ring boot + first-input
window so the HAM clock gate is at 2.4GHz when QP starts.

All matmul operands bf16 (PSUM f32). No row-max subtraction before exp:
scores ~ N(0,1), |scores| <= ~6. Measured rel_l2 ~5e-3 (tolerance 2e-2).
"""
import sys

sys.path.insert(0, "/opt/trn_rl_repo")

import ml_dtypes
import numpy as np

BF16 = ml_dtypes.bfloat16

import concourse.bass as bass
import concourse.tile as tile
from concourse import bacc, bass_utils, mybir

B, S, E, A = 4, 2048, 1024, 1024
SQ = 1024          # queries per core
ET, AT = 8, 8      # 128-tiles of E and A
KT = 16            # 128-tiles of the 2048 keys
KC = 4             # 512-key chunks
QC, QS, AC = 2, 8, 2    # q 512-chunks, q 128-subtiles, a 512-chunks
SCALE = 1.0 / 32.0      # 1/sqrt(A)
NWARM = 19              # PE warm-up matmuls during the initial DMA window

f32 = mybir.dt.float32
bf16 = mybir.dt.bfloat16
ts = bass.ts


def build():
    nc = bacc.Bacc("TRN2", target_bir_lowering=False, debug=False,
                   dynamic_dma_scratch_size=8192)
    Act = mybir.ActivationFunctionType
    Alu = mybir.AluOpType

    # Host-supplied SBUF images (contiguous per-partition rows):
    #   xq_d  [128, et*1024]: xq[p, et*SQ+j]   = query_half^T[et*128+p, j]
    #   wqk_d [128, at*1024]: at-major blocks, wqk[p, at*1024 + et*128 + j]
    #                         = W_qk[et*128+p, at*128+j]
    #   kt_d  [128, kc*4096]: key-chunk-major, kt_d[p, kc*4096 + et*512 + j]
    #                         = key_b^T[et*128+p, kc*512+j]
    #   xv_d  [128, kt*1024]: value rows tiled by key, xv_d[p, kt*1024+e]
    #                         = value_b[kt*128+p, e]
    #   wv_d  [128, et*1024]: wv_d[p, et*1024+j] = Wv[et*128+p, j]
    xq_d = nc.dram_tensor("xq", [128, ET * SQ], bf16, kind="ExternalInput")
    wqk_d = nc.dram_tensor("wqk", [128, AT * SQ], bf16, kind="ExternalInput")
    kt_d = nc.dram_tensor("ktc", [128, KC * ET * 512], bf16,
                          kind="ExternalInput")
    xv_d = nc.dram_tensor("xv", [128, KT * A], bf16, kind="ExternalInput")
    wv_d = nc.dram_tensor("wv", [128, ET * A], bf16, kind="ExternalInput")
    bqkt_d = nc.dram_tensor("bqkt", [128, AT], f32, kind="ExternalInput")
    bvb_d = nc.dram_tensor("bvb", [128, A], f32, kind="ExternalInput")
    ones_d = nc.dram_tensor("ones", [128, 2], f32, kind="ExternalInput")
    out_d = nc.dram_tensor("out", [SQ, A], bf16, kind="ExternalOutput")

    # Long-lived activations as raw (non-pool) SBUF tensors.
    qT = nc.alloc_sbuf_tensor("qT_sb", [128, ET, SQ], bf16).ap()
    kT = nc.alloc_sbuf_tensor("kT_sb", [128, KC, ET, 512], bf16).ap()
    v_sb = nc.alloc_sbuf_tensor("v_sb", [128, KT, A], bf16).ap()
    yT = nc.alloc_sbuf_tensor("yT_sb", [128, ET, SQ], bf16).ap()
    acc = nc.alloc_sbuf_tensor("acc_sb", [128, SQ], f32).ap()
    recip = nc.alloc_sbuf_tensor("recip_sb", [128, QS], f32).ap()
    ones_t = nc.alloc_sbuf_tensor("ones_sb", [128, 2], f32).ap()

    with tile.TileContext(nc) as tc:
        with (
            tc.tile_pool(name="pp512", bufs=2, space="PSUM") as pp512,
            tc.tile_pool(name="pps", bufs=3, space="PSUM") as pps,
        ):
            pe = tc.alloc_tile_pool(name="pe", bufs=1)
            E_t = pe.tile([128, KT, SQ], bf16)  # exp(scores^T) [k, kt, q]
            pwq = tc.alloc_tile_pool(name="pwq", bufs=1)
            pwv = tc.alloc_tile_pool(name="pwv", bufs=1)
            pxq = tc.alloc_tile_pool(name="pxq", bufs=1)
            pcs = tc.alloc_tile_pool(name="pcs", bufs=1)
            pot = tc.alloc_tile_pool(name="pot", bufs=3)

            # ---- PE warm-up: dummy matmuls on a zeroed SBUF scrap while
            # the DMA rings boot (~8us) and the first inputs land, so the
            # HAM clock ramp overlaps the dead startup window.
            nc.vector.memset(qT[:, 0, 0:512], 0.0)
            for i in range(NWARM):
                wps = pp512.tile([128, 512], f32, tag="ps", name="wps")
                nc.tensor.matmul(wps[:], qT[:, 0, 0:128], qT[:, 0, 0:512],
                                 start=True, stop=True)

            # ---- Input stream, ONE queue (Scalar), consumption order ----
            wqk = pwq.tile([128, AT, ET * 128], bf16)
            xq_t = pxq.tile([128, QC, ET, 512], bf16)
            wqkd = wqk_d.ap()
            nc.scalar.dma_start(wqk[:, 0, :], wqkd[:, 0:1024])
            # xq image is qc-major and qc0 arrives in four 256KB
            # et-pair chunks: QP's first matmul waits for ~512KB
            for h in range(4):
                nc.scalar.dma_start(
                    xq_t[:, 0, ts(h, 2), :],
                    xq_d.ap()[:, h * 1024:(h + 1) * 1024].rearrange(
                        "p (et j) -> p et j", j=512))
            for h in range(2):   # qc1 in halves: its first bytes are
                nc.scalar.dma_start(   # consumable 1us sooner
                    xq_t[:, 1, ts(h, 4), :],
                    xq_d.ap()[:, 4096 + h * 2048:4096 + (h + 1) * 2048
                              ].rearrange("p (et j) -> p et j", j=512))
            for at in range(1, AT):
                nc.scalar.dma_start(
                    wqk[:, at, :], wqkd[:, at * 1024:(at + 1) * 1024])
            for kc in range(KC):     # keyT in S-phase chunk order
                nc.scalar.dma_start(
                    kT[:, kc, :, :],
                    kt_d.ap()[:, kc * 4096:(kc + 1) * 4096].rearrange(
                        "p (et j) -> p et j", j=512))
            for h in range(4):       # value in 4-kt chunks (AV1 order)
                nc.scalar.dma_start(
                    v_sb[:, ts(h, 4), :],
                    xv_d.ap()[:, h * 4096:(h + 1) * 4096].rearrange(
                        "p (kt j) -> p kt j", j=A))
            wv = pwv.tile([128, ET, A], bf16)
            for h in range(2):
                nc.scalar.dma_start(
                    wv[:, ts(h, 4), :],
                    wv_d.ap()[:, h * 4096:(h + 1) * 4096].rearrange(
                        "p (b j) -> p b j", j=A))

            # tiny bias constants ride gpsimd (SWDGE), off both big queues
            bqkt = pcs.tile([128, AT], f32, tag="bqkt")
            nc.gpsimd.dma_start(bqkt[:], bqkt_d.ap()[:, :])
            nc.gpsimd.dma_start(ones_t[:], ones_d.ap()[:, :])
            bvb = pcs.tile([128, A], f32, tag="bvb")
            nc.gpsimd.dma_start(bvb[:], bvb_d.ap()[:, :])

            # ---- Phase QP: qT[e', q] = (query @ W_qk + bqk)^T ----
            for at in range(AT):
                ps = pps.tile([128, SQ], f32, tag="psc", name="ps_a")
                for qc in range(QC):   # qc-outer: the first 8-matmul pass
                    for et in range(ET):   # only needs xq's qc=0 half
                        nc.tensor.matmul(
                            ps[:, ts(qc, 512)], wqk[:, at, ts(et, 128)],
                            xq_t[:, qc, et, :],
                            start=(et == 0), stop=(et == ET - 1),
                        )
                nc.vector.tensor_scalar(
                    qT[:, at, :], ps[:], bqkt[:, at:at + 1], None, Alu.add)

            # ---- Phase S: scores^T = kT-tile @ qT -> exp -> E_t; Vector
            #      accumulates softmax denominators in production order ----
            nprod = 0
            for kc in range(KC):
                for ki in range(4):
                    kt = kc * 4 + ki
                    psc = pps.tile([128, SQ], f32, tag="psc", name="psc")
                    for et in range(ET):
                        for qc in range(QC):
                            nc.tensor.matmul(
                                psc[:, ts(qc, 512)],
                                kT[:, kc, et, ts(ki, 128)],
                                qT[:, et, ts(qc, 512)],
                                start=(et == 0), stop=(et == ET - 1),
                            )
                    nc.scalar.activation(
                        E_t[:, kt, :], psc[:], Act.Exp, bias=0.0,
                        scale=SCALE)
                    nprod += 1
                    if nprod == 2:
                        nc.vector.tensor_tensor(
                            acc[:], E_t[:, 0, :], E_t[:, 1, :], Alu.add)
                    elif nprod > 2:
                        nc.vector.tensor_tensor(
                            acc[:], acc[:], E_t[:, kt, :], Alu.add)

            # ---- Phase AV1: yT[e, q] = value^T @ E_t (unnormalized) ----
            for es in range(ET):
                ps = pps.tile([128, SQ], f32, tag="psc", name="ps_y")
                for kt in range(KT):
                    for qc in range(QC):
                        nc.tensor.matmul(
                            ps[:, ts(qc, 512)], v_sb[:, kt, ts(es, 128)],
                            E_t[:, kt, ts(qc, 512)],
                            start=(kt == 0), stop=(kt == KT - 1),
                        )
                nc.vector.tensor_copy(yT[:, es, :], ps[:])

            # denominators: 128-way partition reduction of acc via tiny
            # matmuls with a ones column; tucked between AV1 and AV2 so
            # the PE covers the last acc adds / yT drain
            for dq in range(QS):
                psd = pp512.tile([128, 2], f32, tag="ps", name="psd")
                nc.tensor.matmul(
                    psd[:], acc[:, ts(dq, 128)], ones_t[:],
                    start=True, stop=True)
                nc.vector.reciprocal(recip[:, dq:dq + 1], psd[:, 0:1])

            # ---- Phase AV2: out = (yT^T @ Wv) * recip + bv ----
            for qs in range(QS):
                ps = pps.tile([128, SQ], f32, tag="psc", name="ps_av")
                for et in range(ET):
                    for ac in range(AC):
                        nc.tensor.matmul(
                            ps[:, ts(ac, 512)], yT[:, et, ts(qs, 128)],
                            wv[:, et, ts(ac, 512)],
                            start=(et == 0), stop=(et == ET - 1),
                        )
                ot = pot.tile([128, SQ], bf16, tag="ot", name="ot")
                nhalf = 2 if qs >= QS - 2 else 1
                step = SQ // nhalf
                for h in range(nhalf):
                    sl = slice(h * step, (h + 1) * step)
                    nc.vector.tensor_scalar(
                        ot[:, sl], ps[:, sl], recip[:, qs:qs + 1],
                        None, Alu.mult)
                    nc.vector.tensor_tensor(
                        ot[:, sl], ot[:, sl], bvb[:, sl], Alu.add)
                    nc.sync.dma_start(
                        out_d.ap()[ts(qs, 128), sl], ot[:, sl])

            for p in (pot, pcs, pxq, pwv, pwq, pe):
                p.release()

    nc.compile()
    return nc


_nc_cache = None


def _get_nc():
    global _nc_cache
    if _nc_cache is None:
        _nc_cache = build()
    return _nc_cache


def _img(xT, c0=None, c1=None):
    """[E, n] -> SBUF image [128, 8*n'] (p-major), optionally col-sliced."""
    t = xT.reshape(ET, 128, xT.shape[1]).transpose(1, 0, 2)
    if c0 is None:
        return np.ascontiguousarray(t.reshape(128, -1))
    return np.ascontiguousarray(t[:, :, c0:c1].reshape(128, -1))


def kernel(query, key, value, Wq, bq, Wk, bk, Wv, bv):
    query = np.asarray(query, dtype=np.float32)
    key = np.asarray(key, dtype=np.float32)
    value = np.asarray(value, dtype=np.float32)
    Wq = np.ascontiguousarray(np.asarray(Wq, dtype=np.float32))
    Wk = np.ascontiguousarray(np.asarray(Wk, dtype=np.float32))
    Wv = np.ascontiguousarray(np.asarray(Wv, dtype=np.float32))
    bq = np.asarray(bq, dtype=np.float32)
    bk = np.asarray(bk, dtype=np.float32)
    bv = np.asarray(bv, dtype=np.float32)

    nc = _get_nc()

    # Projection folding (see module docstring): scores row-offsets from
    # bk cancel in softmax, so only W_qk and bqk are needed.
    Wqk16 = (Wq @ Wk.T).astype(BF16)
    bqk = bq @ Wk.T                       # [E]
    Wv16 = Wv.astype(BF16)

    wqk_i = np.concatenate(
        [_img(Wqk16, at * 128, (at + 1) * 128) for at in range(AT)], axis=1)
    wv_i = _img(Wv16)
    bqkt = np.ascontiguousarray(bqk.reshape(AT, 128).T)
    bvb = np.ascontiguousarray(np.broadcast_to(bv, (128, A)))
    ones = np.ones((128, 2), np.float32)

    in_maps = []
    for c in range(8):
        b, h = c // 2, c % 2
        xqT = query[b, h * SQ:(h + 1) * SQ, :].T.astype(BF16)
        keyT = key[b].T.astype(BF16)              # [E, 2048]
        val16 = value[b].astype(BF16)             # [2048, E]
        xq_img = np.concatenate(
            [_img(xqT, qc * 512, (qc + 1) * 512) for qc in range(QC)],
            axis=1)
        kt_img = np.concatenate(
            [_img(keyT, kc * 512, (kc + 1) * 512) for kc in range(KC)],
            axis=1)
        xv_img = np.ascontiguousarray(
            val16.reshape(KT, 128, A).transpose(1, 0, 2).reshape(128, -1))
        in_maps.append({
            "xq": xq_img,
            "wqk": wqk_i,
            "ktc": kt_img,
            "xv": xv_img,
            "wv": wv_i,
            "bqkt": bqkt,
            "bvb": bvb,
            "ones": ones,
        })

    global _last_in_maps
    _last_in_maps = in_maps
    res = bass_utils.run_bass_kernel_spmd(nc, in_maps, core_ids=list(range(8)))

    out = np.empty((B, S, A), np.float32)
    for c in range(8):
        b, h = c // 2, c % 2
        out[b, h * SQ:(h + 1) * SQ, :] = np.asarray(
            res.results[c]["out"], dtype=np.float32)
    return out
